# revision 1
# baseline (speedup 1.0000x reference)
"""Trainium2 Bass kernel for a 4-layer alternating-direction LSTM encoder + FFN.

Problem shapes (hardcoded): B=64, T=512, H=512, F=2048, L=4, gates 4H=2048.

Strategy: data-parallel over batch across 8 cores (8 examples/core). All
on-device tensors live in a transposed layout (feature dim on the 128 SBUF
partitions, (time, batch) on the free dim) so the per-timestep elementwise
work runs at full 128-lane width. Per layer, three phases:

  A) XwT[t] = (Wx^T x_t + b)  for all t   -- one big batched matmul -> DRAM
  B) sequential LSTM recurrence over T steps: gates_t = XwT_t + Wh^T h_{t-1};
     stationary = Wh tiles (bf16, fast-weight-load), moving = h^T [128 x 8].
  C) FFN: a2 = W2^T relu(W1^T h + b1) + b2  -- batched matmuls.

Layer direction flips are handled purely by index order (read-side reversal
of XwT blocks in phase B + reversed staging-slot order), data always stays in
global time order. Host side only reshapes/casts (sharding + layout prep).
"""

import numpy as np
import ml_dtypes
from contextlib import ExitStack

import concourse.bass as bass
from concourse import bacc
import concourse.mybir as mybir
import concourse.tile as tile
from concourse.bass import ds, ts
from concourse.bass_utils import run_bass_kernel_spmd

BF16 = mybir.dt.bfloat16
F32 = mybir.dt.float32
AF = mybir.ActivationFunctionType
ALU = mybir.AluOpType

B, T, H, F, L = 64, 512, 512, 2048, 4
NCORES = 8
BL = B // NCORES          # 8 examples per core
NT = BL * T               # 4096 free-dim columns (t-major: col = t*BL + b)
KH = H // 128             # 4 contraction chunks over H
MG = (4 * H) // 128       # 16 gate-dim m-tiles
KF = F // 128             # 16 contraction chunks over F
TBLK = 32                 # recurrence steps per staged block
NBLK = T // TBLK          # 16 blocks
BODY = 2 * TBLK           # 64 steps per For_i body (2 blocks)
CHUNK = 512               # batched-matmul moving free dim
NCHUNK = NT // CHUNK      # 8

_built = None
DEBUG = False
TRACE = False
last_results = None
import os as _os
REPEAT = int(_os.environ.get("BASS_LSTM_REPEAT", "1"))
SKIP_B = _os.environ.get("BASS_LSTM_SKIP_B") == "1"
B_PE_ONLY = _os.environ.get("BASS_LSTM_B_PE_ONLY") == "1"


def _build_nc():
    nc = bacc.Bacc(None, target_bir_lowering=False)

    xT = nc.declare_dram_parameter("xT", [KH, 128, NT], BF16, isOutput=False)
    wx = nc.declare_dram_parameter("wx", [L, 128, KH, 4 * H], BF16, isOutput=False)
    wh = nc.declare_dram_parameter("wh", [L, 128, KH, 4 * H], BF16, isOutput=False)
    gb = nc.declare_dram_parameter("gb", [L, 128, MG], F32, isOutput=False)
    w1 = nc.declare_dram_parameter("w1", [L, 128, KH, F], BF16, isOutput=False)
    b1v = nc.declare_dram_parameter("b1v", [L, 128, KF], F32, isOutput=False)
    w2 = nc.declare_dram_parameter("w2", [L, 128, KF, H], BF16, isOutput=False)
    b2v = nc.declare_dram_parameter("b2v", [L, 128, KH], F32, isOutput=False)
    out = nc.declare_dram_parameter("out", [KH, 128, NT], F32, isOutput=True)

    # XwT scratch, flattened 2D so all dynamic-offset DMAs stay contiguous:
    # slot s (time-block) at cols [s*4096, (s+1)*4096), m-major inside.
    xwt = nc.dram_tensor("xwt", [128, NBLK * MG * TBLK * BL], BF16)
    # hidden-state round-trip scratch (SBUF dynamic-offset DMA isn't available)
    # cols: (tblock, q, hc, b) -- a staging block is written contiguously
    hts = nc.dram_tensor("hts", [128, NBLK * TBLK * 32], BF16)
    if DEBUG:
        dbg_xwt = nc.declare_dram_parameter("dbg_xwt", [128, NBLK * MG * TBLK * BL], BF16, isOutput=True)
        dbg_hts = nc.declare_dram_parameter("dbg_hts", [128, NBLK * TBLK * 32], BF16, isOutput=True)

    with tile.TileContext(nc) as tc, ExitStack() as ctx:
        wpool = ctx.enter_context(tc.tile_pool(name="weights", bufs=1))
        state = ctx.enter_context(tc.tile_pool(name="state", bufs=1))
        small = ctx.enter_context(tc.tile_pool(name="small", bufs=2))
        xwin = ctx.enter_context(tc.tile_pool(name="xwin", bufs=1))
        tmpa = ctx.enter_context(tc.tile_pool(name="tmpa", bufs=3))
        a1p = ctx.enter_context(tc.tile_pool(name="a1p", bufs=1))
        ewp = ctx.enter_context(tc.tile_pool(name="ewp", bufs=3))
        psum = ctx.enter_context(tc.tile_pool(name="psum", bufs=8, space="PSUM"))

        # Persistent state: two ping-pong sequence buffers (feature-transposed,
        # [128, KH, NT]), the recurrence staging buffer and cell state.
        bufs = [state.tile([128, KH, NT], BF16, tag=f"seq{i}", name=f"seq{i}") for i in range(2)]
        # staging: [carry 32][64 slots x 32]; slot q at cols 32+q*32,
        # col layout within a slot: (hchunk, b)
        stag = state.tile([128, 32 + BODY * 32], BF16, tag="stag")
        cst = [state.tile([128, 32], F32, tag=f"c{i}", name=f"c{i}") for i in range(2)]

        import contextlib
        rep_ctx = tc.For_i(0, REPEAT, 1) if REPEAT > 1 else contextlib.nullcontext()
        with rep_ctx:
            _build_pass(nc, tc, locals())

    nc.finalize()
    return nc


def _build_pass(nc, tc, env):
    xT, wx, wh, gb, w1, b1v, w2, b2v, out = (
        env["xT"], env["wx"], env["wh"], env["gb"], env["w1"],
        env["b1v"], env["w2"], env["b2v"], env["out"])
    xwt, hts = env["xwt"], env["hts"]
    wpool, state, small, xwin, tmpa, a1p, ewp, psum = (
        env["wpool"], env["state"], env["small"], env["xwin"],
        env["tmpa"], env["a1p"], env["ewp"], env["psum"])
    bufs, stag, cst = env["bufs"], env["stag"], env["cst"]
    if DEBUG:
        dbg_xwt, dbg_hts = env["dbg_xwt"], env["dbg_hts"]
    if True:
        for layer in range(L):
            rev = layer % 2 == 1
            buf_in = bufs[0]   # layer input; FFN writes its output back here
            buf_out = bufs[1]  # recurrence hidden states

            # ---- weights + biases for this layer ----
            wx_sb = wpool.tile([128, KH, 4 * H], BF16, tag="wx")
            wh_sb = wpool.tile([128, KH, 4 * H], BF16, tag="wh")
            w1_sb = wpool.tile([128, KH, F], BF16, tag="w1")
            w2_sb = wpool.tile([128, KF, H], BF16, tag="w2")
            nc.sync.dma_start(out=wx_sb, in_=wx[layer])
            nc.sync.dma_start(out=wh_sb, in_=wh[layer])
            nc.sync.dma_start(out=w1_sb, in_=w1[layer])
            nc.sync.dma_start(out=w2_sb, in_=w2[layer])
            gb_sb = small.tile([128, MG], F32, tag="gb")
            b1_sb = small.tile([128, KF], F32, tag="b1")
            b2_sb = small.tile([128, KH], F32, tag="b2")
            nc.sync.dma_start(out=gb_sb, in_=gb[layer])
            nc.sync.dma_start(out=b1_sb, in_=b1v[layer])
            nc.sync.dma_start(out=b2_sb, in_=b2v[layer])

            if layer == 0:
                for k in range(KH):
                    nc.sync.dma_start(out=buf_in[:, k, :], in_=xT[k])

            # ---------------- Phase A: XwT = Wx^T @ in + b ----------------
            for c in range(NCHUNK):
                cols = ds(c * CHUNK, CHUNK)
                for m in range(MG):
                    pt = psum.tile([128, CHUNK], F32, tag="ps")
                    for k in range(KH):
                        nc.tensor.matmul(
                            pt,
                            wx_sb[:, k, ts(m, 128)],
                            buf_in[:, k, cols],
                            start=(k == 0),
                            stop=(k == KH - 1),
                        )
                    sb = tmpa.tile([128, CHUNK], BF16, tag="xa")
                    nc.scalar.activation(sb, pt, AF.Identity, bias=gb_sb[:, m : m + 1])
                    # chunk c covers time-blocks 2c, 2c+1
                    o0 = (2 * c) * 4096 + m * 256
                    o1 = (2 * c + 1) * 4096 + m * 256
                    nc.sync.dma_start(out=xwt[:, ds(o0, 256)], in_=sb[:, 0:256])
                    nc.sync.dma_start(out=xwt[:, ds(o1, 256)], in_=sb[:, 256:512])

            # ---------------- Phase B: recurrence ----------------
            if SKIP_B:
                continue
            nc.vector.memset(stag[:, 0:32], 0.0)
            nc.vector.memset(cst[0], 0.0)

            def slot_w(s):
                # staging slot written by step s (read-side global-time order)
                k, p = divmod(s, TBLK)
                return k * TBLK + (TBLK - 1 - p) if rev else s

            def stag_cols(q):
                return ds(32 + q * 32, 32)

            with tc.For_i(0, NBLK, 2, hint_engines=(mybir.EngineType.PE,)) as jv:
                # one contiguous XwT load covering both blocks of this body
                xwbody = xwin.tile([128, 2 * MG * TBLK * BL], BF16, name="xwbody")
                if rev:
                    base = (NBLK - 2) * 4096 - jv * 4096
                else:
                    base = jv * 4096
                nc.sync.dma_start(out=xwbody, in_=xwt[:, ds(base, 8192)])
                xwv = xwbody.rearrange("p (h m c) -> p h m c", h=2, m=MG)

                for s in range(BODY):
                    blk = s // TBLK
                    pos = s % TBLK
                    scol = (TBLK - 1 - pos) if rev else pos
                    rd_base = (32 + slot_w(s - 1) * 32) if s > 0 else 0

                    # gate matmuls: one psum bank per step, cols (g, hco, b)
                    pt = psum.tile([128, 128], F32, tag="ps", name="ptg")
                    for g in (0, 1, 2, 3):  # i, f, g, o
                        for hco in range(4):
                            m = g * 4 + hco
                            for k in range(KH):
                                nc.tensor.matmul(
                                    pt[:, ds(m * 8, 8)],
                                    wh_sb[:, k, ts(m, 128)],
                                    stag[:, ds(rd_base + k * 8, 8)],
                                    start=(k == 0),
                                    stop=(k == KH - 1),
                                )

                    if B_PE_ONLY:
                        continue
                    half = (1 - blk) if rev else blk
                    gsum = ewp.tile([128, 128], F32, tag="gsum")
                    act = ewp.tile([128, 128], F32, tag="act")
                    # single fused add of all gates: psum + XwT slice
                    nc.vector.tensor_tensor(
                        gsum, pt, xwv[:, half, :, ds(scol * 8, 8)], ALU.add
                    )
                    c_cur, c_nxt = cst[s % 2], cst[(s + 1) % 2]
                    # i,f together; then g; then o
                    nc.scalar.activation(act[:, 0:64], gsum[:, 0:64], AF.Sigmoid)
                    nc.scalar.activation(act[:, 64:96], gsum[:, 64:96], AF.Tanh)
                    fc = ewp.tile([128, 32], F32, tag="fc")
                    nc.vector.tensor_tensor(fc, act[:, 32:64], c_cur, ALU.mult)
                    ig = ewp.tile([128, 32], F32, tag="ig")
                    nc.vector.tensor_tensor(ig, act[:, 0:32], act[:, 64:96], ALU.mult)
                    nc.vector.tensor_tensor(c_nxt, fc, ig, ALU.add)
                    nc.scalar.activation(act[:, 96:128], gsum[:, 96:128], AF.Sigmoid)
                    thc = ewp.tile([128, 32], F32, tag="thc")
                    nc.scalar.activation(thc, c_nxt, AF.Tanh)
                    nc.vector.tensor_tensor(
                        stag[:, stag_cols(slot_w(s))], act[:, 96:128], thc, ALU.mult
                    )

                # write the two blocks of h to the DRAM scratch (contiguous)
                for blk in range(2 if not B_PE_ONLY else 0):
                    if rev:
                        dst = ds((NBLK - 1) * 1024 - (jv + blk) * 1024, 1024)
                    else:
                        dst = ds((jv + blk) * 1024, 1024)
                    nc.sync.dma_start(
                        out=hts[:, dst], in_=stag[:, ds(32 + blk * 1024, 1024)]
                    )
                # carry h_{last} into cols [0:32] for the next body
                if not B_PE_ONLY:
                    nc.vector.tensor_copy(
                        stag[:, 0:32], stag[:, stag_cols(slot_w(BODY - 1))]
                    )

            if DEBUG and layer == 0:
                nc.sync.dma_start(out=dbg_xwt[:, :], in_=xwt[:, :])
                nc.sync.dma_start(out=dbg_hts[:, :], in_=hts[:, :])

            # ---------------- Phase C: FFN ----------------
            last = layer == L - 1
            hv = hts.rearrange("p (n q h b) -> p n q h b", q=TBLK, h=KH, b=BL)
            for c in range(NCHUNK):
                cols = ds(c * CHUNK, CHUNK)
                for hc in range(KH):
                    nc.sync.dma_start(
                        out=buf_out[:, hc, cols],
                        in_=hv[:, 2 * c : 2 * c + 2, :, hc, :],
                    )
                a1 = a1p.tile([128, KF, CHUNK], BF16, tag="a1")
                for m in range(KF):
                    pt = psum.tile([128, CHUNK], F32, tag="ps")
                    for k in range(KH):
                        nc.tensor.matmul(
                            pt,
                            w1_sb[:, k, ts(m, 128)],
                            buf_out[:, k, cols],
                            start=(k == 0),
                            stop=(k == KH - 1),
                        )
                    nc.scalar.activation(
                        a1[:, m, :], pt, AF.Relu, bias=b1_sb[:, m : m + 1]
                    )
                for mo in range(KH):
                    pt = psum.tile([128, CHUNK], F32, tag="ps")
                    for k in range(KF):
                        nc.tensor.matmul(
                            pt,
                            w2_sb[:, k, ts(mo, 128)],
                            a1[:, k, :],
                            start=(k == 0),
                            stop=(k == KF - 1),
                        )
                    if last:
                        ot = tmpa.tile([128, CHUNK], F32, tag="oc")
                        nc.scalar.activation(ot, pt, AF.Identity, bias=b2_sb[:, mo : mo + 1])
                        nc.sync.dma_start(out=out[mo, :, cols], in_=ot)
                    else:
                        nc.scalar.activation(
                            buf_in[:, mo, cols], pt, AF.Identity,
                            bias=b2_sb[:, mo : mo + 1],
                        )


def _get_nc():
    global _built
    if _built is None:
        _built = _build_nc()
    return _built


def kernel(**inputs):
    x = np.asarray(inputs["x"], np.float32)
    Wx = np.asarray(inputs["Wx"], np.float32)
    Wh = np.asarray(inputs["Wh"], np.float32)
    b = np.asarray(inputs["b"], np.float32)
    W1 = np.asarray(inputs["W1"], np.float32)
    b1 = np.asarray(inputs["b1"], np.float32)
    W2 = np.asarray(inputs["W2"], np.float32)
    b2 = np.asarray(inputs["b2"], np.float32)

    bf = ml_dtypes.bfloat16
    wx_h = np.ascontiguousarray(Wx.reshape(L, KH, 128, 4 * H).transpose(0, 2, 1, 3)).astype(bf)
    wh_h = np.ascontiguousarray(Wh.reshape(L, KH, 128, 4 * H).transpose(0, 2, 1, 3)).astype(bf)
    gb_h = np.ascontiguousarray(b.reshape(L, MG, 128).transpose(0, 2, 1)).astype(np.float32)
    w1_h = np.ascontiguousarray(W1.reshape(L, KH, 128, F).transpose(0, 2, 1, 3)).astype(bf)
    b1_h = np.ascontiguousarray(b1.reshape(L, KF, 128).transpose(0, 2, 1)).astype(np.float32)
    w2_h = np.ascontiguousarray(W2.reshape(L, KF, 128, H).transpose(0, 2, 1, 3)).astype(bf)
    b2_h = np.ascontiguousarray(b2.reshape(L, KH, 128).transpose(0, 2, 1)).astype(np.float32)

    in_maps = []
    for c in range(NCORES):
        xc = x[c * BL : (c + 1) * BL]  # [BL, T, H]
        # xT[hc, p, t*BL + b] = xc[b, t, hc*128+p]
        xt = np.ascontiguousarray(
            xc.reshape(BL, T, KH, 128).transpose(2, 3, 1, 0).reshape(KH, 128, NT)
        ).astype(bf)
        in_maps.append(
            dict(
                xT=xt, wx=wx_h, wh=wh_h, gb=gb_h,
                w1=w1_h, b1v=b1_h, w2=w2_h, b2v=b2_h,
            )
        )

    nc = _get_nc()
    global last_results
    import kernel as _K
    _K.kernel_prepped_maps = in_maps
    kr = run_bass_kernel_spmd(
        nc, in_maps, core_ids=list(range(NCORES)), trace=TRACE
    )
    last_results = kr
    res = kr.results

    outp = np.empty((B, T, H), np.float32)
    for c in range(NCORES):
        oc = res[c]["out"]  # [KH, 128, NT] f32
        outp[c * BL : (c + 1) * BL] = (
            oc.reshape(KH, 128, T, BL).transpose(3, 2, 0, 1).reshape(BL, T, H)
        )
    if DEBUG:
        kernel.last_debug = res
    return outp


if __name__ == "__main__":
    _get_nc()
    print("build ok")

